# revision 21
# baseline (speedup 1.0000x reference)
"""DSAttention TRN2 Bass kernel.

Reference (per batch b, head h, branch):
    z[l,s] = (q[l]·k[s]) * tau[b]/8 + delta[b,s]/8        (causal: s <= l)
    A = softmax_s(z);  O = A @ V
    out = m*O_edit + (1-m)*O_null,  m = soft_mask[b,l]

Sharding: B*H = 16 (b,h) slices -> 8 cores x 2 heads. Same SPMD program on
every core; core c gets b = c//4, heads 2*(c%4), 2*(c%4)+1.

Per-core algorithm (transposed-score flash attention):
  - Host pre-packs per head: qkt = [[Q^T; Q^T] | [K_e^T; K_n^T]] [128, 2L]
    in bf16 so the two branches' QK^T matmuls run row-packed
    (tile_position (0,0)/(64,0)) concurrently on the PE array, and V with a
    ones column appended ([2, S, 65], f32r) in natural layout.  All
    constants ride in one blob DMA; inputs are 2 DMAs per head; outputs
    are 1 DMA per 512-row L-chunk (SWDGE fixed cost ~1us per dma_start).
  - scores^T tile [s:128, l:<=512] per (S-tile, L-chunk), trimmed to the
    causal region. exp runs split across TWO engines to break the ACT
    bottleneck: tiles with (index % DVE_MOD) in DVE_RES use a Schraudolph
    fast-exp on DVE (exp(z) ~= bitcast_f32(int32(A*z+B)), +-3% per weight
    pre-normalization, ~0.2% end-to-end after softmax cancellation); the
    rest use the exact ACT exp with fused scale=tau/8, bias=delta_s/8.
    Diagonal 128x128 blocks masked post-exp with a host triangular 0/1
    tile on GPSIMD (keeps DVE free).
  - O^T[65, 512] accumulates AV matmuls over S-tiles (V augmented with the
    ones column => row 64 of O^T is the softmax denominator).  The js loop
    is software-pipelined: QK/exp run one step ahead of AV in PE program
    order so PSUM WAR waits never block the in-order PE queue.
  - epilogue: PSUM->SBUF copies split across ACT and DVE; batched
    PE-transposes into [128, 4, 65] PSUM tiles; ONE DVE divide per branch
    for the 4 sub-tiles' m/den scalars; 2 blend ops per 128-row tile into
    a [128, 4, 64] staging tile; one DMA per chunk.  Each chunk's
    epilogue tail is deferred into the next chunk's js loop so the PE
    never idles at chunk boundaries.

REPEAT > 1 wraps the whole per-core program in a hardware For_i loop; used
by the timing harness to measure per-iteration HW time from wall-clock
deltas (transfers cancel).
"""

import contextlib

import numpy as np
import ml_dtypes

import concourse.bass as bass
import concourse.tile as tile
from concourse import bacc, mybir
from concourse.bass_utils import run_bass_kernel_spmd

B, L, S, H, E, D = 2, 2048, 2048, 8, 64, 64
NCORES = 8
HPC = 2            # heads per core
NT = 16            # 128-row tiles in 2048
LCH = 4            # 512-wide L chunks
F32 = mybir.dt.float32
F32R = mybir.dt.float32r
BF16 = mybir.dt.bfloat16
I16 = mybir.dt.int16
EXPF = mybir.ActivationFunctionType.Exp
MUL = mybir.AluOpType.mult
ADD = mybir.AluOpType.add
DIV = mybir.AluOpType.divide

# Schraudolph fast-exp (bf16): exp(z) ~= bitcast_bf16(int16(EXP_A*z + EXP_B))
EXP_A = float(2**7 / np.log(2))
EXP_B = 16250.4914

# const blob column layout
C_IDENT = 0
C_MASK = 128
C_CD = 256
C_CDD = 272
C_ST = 288
C_STD = 289
C_NCOL = 290

TRACE = False
LAST_EXEC_NS = None
PTS_BUFS = 8
OSB_BUFS = 3
OB_BUFS = 3
REPEAT = 1
# exp-engine split: tile t goes to DVE iff t % DVE_MOD in DVE_RES
DVE_MOD = 7
DVE_RES = (3, 6)
MASK_GPS = True
COPY_ACT = 1       # how many of the 2 PSUM->SBUF copies run on ACT

_NC = None


def _build():
    nc = bacc.Bacc("TRN2")
    qkt_p = nc.declare_dram_parameter("qkt", [HPC, 128, 2 * L], BF16,
                                      isOutput=False)
    vv_p = nc.declare_dram_parameter("vv", [HPC, 2, S, D + 1], BF16,
                                     isOutput=False)
    cb_p = nc.declare_dram_parameter("cblob", [128, C_NCOL], F32,
                                     isOutput=False)
    mk_p = nc.declare_dram_parameter("mask", [128, 128], BF16,
                                     isOutput=False)
    mtb_p = nc.declare_dram_parameter("mtb", [128, NT, 2], F32,
                                      isOutput=False)
    out_p = nc.declare_dram_parameter("out", [HPC, L, D], F32, isOutput=True)
    params = (qkt_p, vv_p, cb_p, mk_p, mtb_p, out_p)

    with tile.TileContext(nc) as tc:
        with (
            tc.tile_pool(name="const", bufs=1) as const,
            tc.tile_pool(name="big", bufs=2) as big,
            tc.tile_pool(name="pts", bufs=PTS_BUFS) as pts,
            tc.tile_pool(name="osb", bufs=OSB_BUFS) as osb,
            tc.tile_pool(name="sml", bufs=8) as sml,
            tc.tile_pool(name="ob", bufs=OB_BUFS) as ob,
            tc.tile_pool(name="ps_pt", bufs=2, space="PSUM") as ps_pt,
            tc.tile_pool(name="ps_oac", bufs=1, space="PSUM") as ps_oac,
            tc.tile_pool(name="ps_tr", bufs=1, space="PSUM") as ps_tr,
        ):
            pools = (const, big, pts, osb, sml, ob, ps_pt, ps_oac, ps_tr)
            rep = (
                tc.For_i(0, REPEAT, 1)
                if REPEAT > 1
                else contextlib.nullcontext()
            )
            with rep:
                _body(nc, pools, params)
    if not nc.is_finalized():
        nc.finalize()
    return nc


def _body(nc, pools, params):
    const, big, pts, osb, sml, ob, ps_pt, ps_oac, ps_tr = pools
    qkt_p, vv_p, cb_p, mk_p, mtb_p, out_p = params


    tiles = {}
    qkts = {}
    for bh in range(HPC):
        qkt = big.tile([128, 2 * L], BF16, tag="qkt")
        nc.sync.dma_start(out=qkt, in_=qkt_p[bh])
        qkts[bh] = qkt
    cb = const.tile([128, C_NCOL], F32, tag="cblob")
    nc.scalar.dma_start(out=cb, in_=cb_p[:])
    mask = const.tile([128, 128], BF16, tag="mask")
    nc.scalar.dma_start(out=mask, in_=mk_p[:])
    mtb = const.tile([128, NT, 2], F32, tag="mtb")
    nc.scalar.dma_start(out=mtb, in_=mtb_p[:])
    for bh in range(HPC):
        vv = big.tile([128, 2, NT, D + 1], BF16, tag="vv")
        nc.sync.dma_start(
            out=vv, in_=vv_p[bh].rearrange("b (t p) d -> p b t d", p=128)
        )
        qkt = qkts[bh]
        tiles[bh] = (qkt[:, 0:L], qkt[:, L : 2 * L], vv[:, 0], vv[:, 1])

    ident = cb[:, C_IDENT : C_IDENT + 128]
    cdelta = cb[:, C_CD : C_CD + NT]
    cdD = cb[:, C_CDD : C_CDD + NT]
    st = cb[:, C_ST : C_ST + 1]
    stD = cb[:, C_STD : C_STD + 1]

    def epi_a(bh, lc, oe_sb, on_sb):
        tr_e4 = ps_tr.tile([128, 4, D + 1], F32, tag="tre")
        tr_n4 = ps_tr.tile([128, 4, D + 1], F32, tag="trn")
        for t4 in range(4):
            csl = slice(128 * t4, 128 * t4 + 128)
            nc.tensor.transpose(
                tr_e4[:, t4, :], oe_sb[:, csl], ident[0 : D + 1, 0 : D + 1]
            )
            nc.tensor.transpose(
                tr_n4[:, t4, :], on_sb[:, csl], ident[0 : D + 1, 0 : D + 1]
            )
        rec4 = sml.tile([128, 4, 2], F32, tag="rec")
        nc.vector.reciprocal(rec4[:, :, 0:1], tr_e4[:, :, D : D + 1])
        nc.vector.reciprocal(rec4[:, :, 1:2], tr_n4[:, :, D : D + 1])
        ss4 = sml.tile([128, 4, 2], F32, tag="ss")
        nc.vector.tensor_mul(ss4, mtb[:, 4 * lc : 4 * lc + 4, :], rec4)
        obuf = ob.tile([128, 4, D], F32, tag="ob")
        return (bh, lc, tr_e4, tr_n4, ss4, obuf)

    def epi_blend(state, t4s):
        bh, lc, tr_e4, tr_n4, ss4, obuf = state
        for t4 in t4s:
            nc.vector.tensor_scalar_mul(
                obuf[:, t4, :], tr_e4[:, t4, 0:D], ss4[:, t4, 0:1]
            )
            nc.vector.scalar_tensor_tensor(
                out=obuf[:, t4, :], in0=tr_n4[:, t4, 0:D],
                scalar=ss4[:, t4, 1:2], in1=obuf[:, t4, :], op0=MUL, op1=ADD,
            )

    def epi_dma(state):
        bh, lc, tr_e4, tr_n4, ss4, obuf = state
        nc.sync.dma_start(
            out=out_p[bh].rearrange("(t p) d -> p t d", p=128)[
                :, 4 * lc : 4 * lc + 4, :
            ],
            in_=obuf,
        )

    def epilogue_part2(bh, lc, oe_sb, on_sb):
        state = epi_a(bh, lc, oe_sb, on_sb)
        epi_blend(state, (0, 1, 2, 3))
        epi_dma(state)

    def emit_copy_e(chunk):
        bh, lc, oac_e, oac_n = chunk
        oe_sb = osb.tile([D + 1, 512], F32, tag="oesb")
        if COPY_ACT >= 1:
            nc.scalar.copy(out=oe_sb, in_=oac_e)
        else:
            nc.vector.tensor_copy(out=oe_sb, in_=oac_e)
        return (bh, lc, oac_n, oe_sb)

    def emit_copy_n(state):
        bh, lc, oac_n, oe_sb = state
        on_sb = osb.tile([D + 1, 512], F32, tag="onsb")
        if COPY_ACT >= 2:
            nc.scalar.copy(out=on_sb, in_=oac_n)
        else:
            nc.vector.tensor_copy(out=on_sb, in_=oac_n)
        return (bh, lc, oe_sb, on_sb)

    # Flatten all (bh, lc, js) into one global software-pipelined stream:
    # QK/exp for step i issue SKEW steps ahead of AV for step i, across
    # chunk boundaries, so the in-order PE queue never drains on the
    # exp(last)->mask->AV(last)->copy chain at a chunk end.
    steps = []  # (bh, lc, js, njs)
    for bh in range(HPC):
        for lc in range(LCH):
            njs = 4 * lc + 4
            for js in range(njs):
                steps.append((bh, lc, js, njs))
    T = len(steps)
    SKEW = 2

    oacs = {}        # (bh, lc) -> (oac_e, oac_n)
    stash = {}       # step idx -> (pt_sb, off)
    copied = {}      # step idx at which to run copies -> chunk key
    epil = {}        # step idx at which to run epilogue -> copies result
    tcount = 0

    for i in range(T + SKEW):
        if i < T:
            bh, lc, js, njs = steps[i]
            qt, kt, ve, vn = tiles[bh]
            lcb = 512 * lc
            off = max(0, 128 * js - lcb)
            sb = 128 * js
            lsl = slice(lcb + off, lcb + 512)
            pt_ps = ps_pt.tile([128, 2, 512], F32, tag="pt")
            nc.tensor.matmul(
                pt_ps[:, 0, off:512],
                kt[0:64, sb : sb + 128],
                qt[0:64, lsl],
                start=True, stop=True, tile_position=(0, 0),
            )
            nc.tensor.matmul(
                pt_ps[:, 1, off:512],
                kt[64:128, sb : sb + 128],
                qt[64:128, lsl],
                start=True, stop=True, tile_position=(64, 0),
            )
            pt_sb = pts.tile([128, 2, 512], BF16, tag="ptsb")
            use_dve = (tcount % DVE_MOD) in DVE_RES
            tcount += 1
            if use_dve:
                nc.vector.tensor_scalar(
                    out=pt_sb[:, :, off:512].bitcast(I16),
                    in0=pt_ps[:, :, off:512],
                    scalar1=stD,
                    scalar2=cdD[:, js : js + 1],
                    op0=MUL,
                    op1=ADD,
                )
            else:
                nc.scalar.activation(
                    out=pt_sb[:, :, off:512],
                    in_=pt_ps[:, :, off:512],
                    func=EXPF,
                    bias=cdelta[:, js : js + 1],
                    scale=st,
                )
            if sb >= lcb:  # diagonal tile: mask l < s
                _meng = nc.gpsimd if MASK_GPS else nc.vector
                for br in range(2):
                    _meng.tensor_mul(
                        pt_sb[:, br, off : off + 128],
                        pt_sb[:, br, off : off + 128],
                        mask,
                    )
            stash[i] = (pt_sb, off)

        j = i - SKEW
        if j < 0:
            continue
        bh, lc, js, njs = steps[j]
        qt, kt, ve, vn = tiles[bh]
        pt_sb, off = stash.pop(j)
        if js == 0:
            oac_e = ps_oac.tile([D + 1, 512], F32, tag="oe")
            oac_n = ps_oac.tile([D + 1, 512], F32, tag="on")
            oacs[(bh, lc)] = (oac_e, oac_n)
        oac_e, oac_n = oacs[(bh, lc)]
        last = js == njs - 1
        nc.tensor.matmul(
            oac_e[:, off:512],
            ve[:, js, :],
            pt_sb[:, 0, off:512],
            start=(js == 0), stop=last,
        )
        nc.tensor.matmul(
            oac_n[:, off:512],
            vn[:, js, :],
            pt_sb[:, 1, off:512],
            start=(js == 0), stop=last,
        )
        if last:
            half = emit_copy_e((bh, lc, oac_e, oac_n))
            del oacs[(bh, lc)]
            if j + 5 < T:
                epil[j + 1] = ("cn", half)
            else:
                epilogue_part2(*emit_copy_n(half))
        while j in epil:
            kind, arg = epil.pop(j)
            if kind == "cn":
                epil[j + 1] = ("a", emit_copy_n(arg))
            elif kind == "a":
                st8 = epi_a(*arg)
                epil[j + 1] = ("b1", st8)
            elif kind == "b1":
                epi_blend(arg, (0, 1))
                epil[j + 1] = ("b2", arg)
            else:
                epi_blend(arg, (2, 3))
                epi_dma(arg)
    for _j in sorted(epil):
        kind, arg = epil[_j]
        if kind == "cn":
            st8 = epi_a(*emit_copy_n(arg))
            epi_blend(st8, (0, 1, 2, 3))
            epi_dma(st8)
        elif kind == "a":
            st8 = epi_a(*arg)
            epi_blend(st8, (0, 1, 2, 3))
            epi_dma(st8)
        elif kind == "b1":
            epi_blend(arg, (0, 1, 2, 3))
            epi_dma(arg)
        else:
            epi_blend(arg, (2, 3))
            epi_dma(arg)


def _host_in_maps(queries, keys, values, keys_null, values_null, tau, delta,
                  soft_mask):
    ident = np.eye(128, dtype=np.float32)
    mask = np.triu(np.ones((128, 128), dtype=np.float32))

    in_maps = []
    for c in range(NCORES):
        b, h0 = c // 4, HPC * (c % 4)
        qkt = np.empty((HPC, 128, 2 * L), ml_dtypes.bfloat16)
        vv = np.empty((HPC, 2, S, D + 1), ml_dtypes.bfloat16)
        for bh in range(HPC):
            h = h0 + bh
            qT = queries[b, :, h, :].T.astype(ml_dtypes.bfloat16)  # [E, L]
            qkt[bh, 0:64, 0:L] = qT
            qkt[bh, 64:128, 0:L] = qT
            qkt[bh, 0:64, L:] = keys[b, :, h, :].T.astype(ml_dtypes.bfloat16)
            qkt[bh, 64:128, L:] = keys_null[b, :, h, :].T.astype(
                ml_dtypes.bfloat16
            )
            vv[bh, 0, :, 0:D] = values[b, :, h, :]
            vv[bh, 0, :, D] = 1.0
            vv[bh, 1, :, 0:D] = values_null[b, :, h, :]
            vv[bh, 1, :, D] = 1.0
        m_t = np.ascontiguousarray(soft_mask[b].reshape(NT, 128).T)
        cdelta = np.ascontiguousarray((delta[b] / 8.0).reshape(NT, 128).T)
        cblob = np.zeros((128, C_NCOL), np.float32)
        cblob[:, C_IDENT : C_IDENT + 128] = ident
        cblob[:, C_CD : C_CD + NT] = cdelta
        cblob[:, C_CDD : C_CDD + NT] = (EXP_A * cdelta + EXP_B).astype(
            np.float32
        )
        cblob[:, C_ST] = tau[b, 0] / 8.0
        cblob[:, C_STD] = EXP_A * tau[b, 0] / 8.0
        mtb = np.empty((128, NT, 2), np.float32)
        mtb[:, :, 0] = m_t
        mtb[:, :, 1] = 1.0 - m_t
        in_maps.append(dict(qkt=qkt, vv=vv, cblob=cblob, mtb=mtb,
                            mask=mask.astype(ml_dtypes.bfloat16)))
    return in_maps


def kernel(queries, keys, values, keys_null, values_null, tau, delta, soft_mask):
    global _NC, LAST_EXEC_NS
    queries = np.asarray(queries, dtype=np.float32)
    keys = np.asarray(keys, dtype=np.float32)
    values = np.asarray(values, dtype=np.float32)
    keys_null = np.asarray(keys_null, dtype=np.float32)
    values_null = np.asarray(values_null, dtype=np.float32)
    tau = np.asarray(tau, dtype=np.float32)
    delta = np.asarray(delta, dtype=np.float32)
    soft_mask = np.asarray(soft_mask, dtype=np.float32)

    if _NC is None:
        _NC = _build()

    in_maps = _host_in_maps(
        queries, keys, values, keys_null, values_null, tau, delta, soft_mask
    )
    res = run_bass_kernel_spmd(
        _NC, in_maps, core_ids=list(range(NCORES)), trace=TRACE
    )
    LAST_EXEC_NS = res.exec_time_ns

    out = np.empty((B, L, H, D), np.float32)
    for c in range(NCORES):
        b, h0 = c // 4, HPC * (c % 4)
        out[b, :, h0 : h0 + HPC, :] = res.results[c]["out"].transpose(1, 0, 2)
    return out


# revision 22
# speedup vs baseline: 1.1600x; 1.1600x over previous
"""DSAttention TRN2 Bass kernel.

Reference (per batch b, head h, branch):
    z[l,s] = (q[l]·k[s]) * tau[b]/8 + delta[b,s]/8        (causal: s <= l)
    A = softmax_s(z);  O = A @ V
    out = m*O_edit + (1-m)*O_null,  m = soft_mask[b,l]

Sharding: B*H = 16 (b,h) slices -> 8 cores x 2 heads. Same SPMD program on
every core; core c gets b = c//4, heads 2*(c%4), 2*(c%4)+1.

Per-core algorithm (transposed-score flash attention):
  - Host pre-packs per head: qkt = [[Q^T; Q^T] | [K_e^T; K_n^T]] [128, 2L]
    in bf16 so the two branches' QK^T matmuls run row-packed
    (tile_position (0,0)/(64,0)) concurrently on the PE array, and V with a
    ones column appended ([2, S, 65], f32r) in natural layout.  All
    constants ride in one blob DMA; inputs are 2 DMAs per head; outputs
    are 1 DMA per 512-row L-chunk (SWDGE fixed cost ~1us per dma_start).
  - scores^T tile [s:128, l:<=512] per (S-tile, L-chunk), trimmed to the
    causal region. exp runs split across TWO engines to break the ACT
    bottleneck: tiles with (index % DVE_MOD) in DVE_RES use a Schraudolph
    fast-exp on DVE (exp(z) ~= bitcast_f32(int32(A*z+B)), +-3% per weight
    pre-normalization, ~0.2% end-to-end after softmax cancellation); the
    rest use the exact ACT exp with fused scale=tau/8, bias=delta_s/8.
    Diagonal 128x128 blocks masked post-exp with a host triangular 0/1
    tile on GPSIMD (keeps DVE free).
  - O^T[65, 512] accumulates AV matmuls over S-tiles (V augmented with the
    ones column => row 64 of O^T is the softmax denominator).  The js loop
    is software-pipelined: QK/exp run one step ahead of AV in PE program
    order so PSUM WAR waits never block the in-order PE queue.
  - epilogue: PSUM->SBUF copies split across ACT and DVE; batched
    PE-transposes into [128, 4, 65] PSUM tiles; ONE DVE divide per branch
    for the 4 sub-tiles' m/den scalars; 2 blend ops per 128-row tile into
    a [128, 4, 64] staging tile; one DMA per chunk.  Each chunk's
    epilogue tail is deferred into the next chunk's js loop so the PE
    never idles at chunk boundaries.

REPEAT > 1 wraps the whole per-core program in a hardware For_i loop; used
by the timing harness to measure per-iteration HW time from wall-clock
deltas (transfers cancel).
"""

import contextlib

import numpy as np
import ml_dtypes

import concourse.bass as bass
import concourse.tile as tile
from concourse import bacc, mybir
from concourse.bass_utils import run_bass_kernel_spmd

B, L, S, H, E, D = 2, 2048, 2048, 8, 64, 64
NCORES = 8
HPC = 2            # heads per core
NT = 16            # 128-row tiles in 2048
LCH = 4            # 512-wide L chunks
F32 = mybir.dt.float32
F32R = mybir.dt.float32r
BF16 = mybir.dt.bfloat16
I16 = mybir.dt.int16
EXPF = mybir.ActivationFunctionType.Exp
MUL = mybir.AluOpType.mult
ADD = mybir.AluOpType.add
DIV = mybir.AluOpType.divide

# Schraudolph fast-exp (bf16): exp(z) ~= bitcast_bf16(int16(EXP_A*z + EXP_B))
EXP_A = float(2**7 / np.log(2))
EXP_B = 16250.4914

# const blob column layout
C_IDENT = 0
C_MASK = 128
C_CD = 256
C_CDD = 272
C_ST = 288
C_STD = 289
C_NCOL = 290

TRACE = False
LAST_EXEC_NS = None
PTS_BUFS = 8
OSB_BUFS = 3
OB_BUFS = 3
REPEAT = 1
# exp-engine split: tile t goes to DVE iff t % DVE_MOD in DVE_RES
DVE_MOD = 7
DVE_RES = (3, 6)
MASK_GPS = True
COPY_ACT = 1       # how many of the 2 PSUM->SBUF copies run on ACT

_NC = None


def _build():
    nc = bacc.Bacc("TRN2")
    qkt_p = nc.declare_dram_parameter("qkt", [HPC, 128, 2 * L], BF16,
                                      isOutput=False)
    vv_p = nc.declare_dram_parameter("vv", [HPC, 2, S, D + 1], BF16,
                                     isOutput=False)
    cb_p = nc.declare_dram_parameter("cblob", [128, C_NCOL], F32,
                                     isOutput=False)
    mk_p = nc.declare_dram_parameter("mask", [128, 128], BF16,
                                     isOutput=False)
    mtb_p = nc.declare_dram_parameter("mtb", [128, NT, 2], F32,
                                      isOutput=False)
    out_p = nc.declare_dram_parameter("out", [HPC, L, D], F32, isOutput=True)
    params = (qkt_p, vv_p, cb_p, mk_p, mtb_p, out_p)

    with tile.TileContext(nc) as tc:
        with (
            tc.tile_pool(name="const", bufs=1) as const,
            tc.tile_pool(name="big", bufs=2) as big,
            tc.tile_pool(name="pts", bufs=PTS_BUFS) as pts,
            tc.tile_pool(name="osb", bufs=OSB_BUFS) as osb,
            tc.tile_pool(name="sml", bufs=8) as sml,
            tc.tile_pool(name="ob", bufs=OB_BUFS) as ob,
            tc.tile_pool(name="ps_pt", bufs=2, space="PSUM") as ps_pt,
            tc.tile_pool(name="ps_oac", bufs=1, space="PSUM") as ps_oac,
            tc.tile_pool(name="ps_tr", bufs=1, space="PSUM") as ps_tr,
        ):
            pools = (const, big, pts, osb, sml, ob, ps_pt, ps_oac, ps_tr)
            rep = (
                tc.For_i(0, REPEAT, 1)
                if REPEAT > 1
                else contextlib.nullcontext()
            )
            with rep:
                _body(nc, pools, params)
    if not nc.is_finalized():
        nc.finalize()
    return nc


def _body(nc, pools, params):
    const, big, pts, osb, sml, ob, ps_pt, ps_oac, ps_tr = pools
    qkt_p, vv_p, cb_p, mk_p, mtb_p, out_p = params


    cb = const.tile([128, C_NCOL], F32, tag="cblob")
    nc.sync.dma_start(out=cb, in_=cb_p[:])
    mtb = const.tile([128, NT, 2], F32, tag="mtb")
    nc.sync.dma_start(out=mtb, in_=mtb_p[:])
    mask = const.tile([128, 128], BF16, tag="mask")
    nc.sync.dma_start(out=mask, in_=mk_p[:])

    tiles = {}
    for bh in range(HPC):
        qkt = big.tile([128, 2 * L], BF16, tag="qkt")
        nc.sync.dma_start(out=qkt, in_=qkt_p[bh])
        vv = big.tile([128, 2, NT, D + 1], BF16, tag="vv")
        nc.sync.dma_start(
            out=vv, in_=vv_p[bh].rearrange("b (t p) d -> p b t d", p=128)
        )
        tiles[bh] = (qkt[:, 0:L], qkt[:, L : 2 * L], vv[:, 0], vv[:, 1])

    ident = cb[:, C_IDENT : C_IDENT + 128]
    cdelta = cb[:, C_CD : C_CD + NT]
    cdD = cb[:, C_CDD : C_CDD + NT]
    st = cb[:, C_ST : C_ST + 1]
    stD = cb[:, C_STD : C_STD + 1]

    def epi_a(bh, lc, oe_sb, on_sb):
        tr_e4 = ps_tr.tile([128, 4, D + 1], F32, tag="tre")
        tr_n4 = ps_tr.tile([128, 4, D + 1], F32, tag="trn")
        for t4 in range(4):
            csl = slice(128 * t4, 128 * t4 + 128)
            nc.tensor.transpose(
                tr_e4[:, t4, :], oe_sb[:, csl], ident[0 : D + 1, 0 : D + 1]
            )
            nc.tensor.transpose(
                tr_n4[:, t4, :], on_sb[:, csl], ident[0 : D + 1, 0 : D + 1]
            )
        rec4 = sml.tile([128, 4, 2], F32, tag="rec")
        nc.vector.reciprocal(rec4[:, :, 0:1], tr_e4[:, :, D : D + 1])
        nc.vector.reciprocal(rec4[:, :, 1:2], tr_n4[:, :, D : D + 1])
        ss4 = sml.tile([128, 4, 2], F32, tag="ss")
        nc.vector.tensor_mul(ss4, mtb[:, 4 * lc : 4 * lc + 4, :], rec4)
        obuf = ob.tile([128, 4, D], F32, tag="ob")
        return (bh, lc, tr_e4, tr_n4, ss4, obuf)

    def epi_blend(state, t4s):
        bh, lc, tr_e4, tr_n4, ss4, obuf = state
        for t4 in t4s:
            nc.vector.tensor_scalar_mul(
                obuf[:, t4, :], tr_e4[:, t4, 0:D], ss4[:, t4, 0:1]
            )
            nc.vector.scalar_tensor_tensor(
                out=obuf[:, t4, :], in0=tr_n4[:, t4, 0:D],
                scalar=ss4[:, t4, 1:2], in1=obuf[:, t4, :], op0=MUL, op1=ADD,
            )

    def epi_dma(state):
        bh, lc, tr_e4, tr_n4, ss4, obuf = state
        nc.sync.dma_start(
            out=out_p[bh].rearrange("(t p) d -> p t d", p=128)[
                :, 4 * lc : 4 * lc + 4, :
            ],
            in_=obuf,
        )

    def epilogue_part2(bh, lc, oe_sb, on_sb):
        state = epi_a(bh, lc, oe_sb, on_sb)
        epi_blend(state, (0, 1, 2, 3))
        epi_dma(state)

    def emit_copies(chunk):
        bh, lc, oac_e, oac_n = chunk
        oe_sb = osb.tile([D + 1, 512], F32, tag="oesb")
        if COPY_ACT >= 1:
            nc.scalar.copy(out=oe_sb, in_=oac_e)
        else:
            nc.vector.tensor_copy(out=oe_sb, in_=oac_e)
        on_sb = osb.tile([D + 1, 512], F32, tag="onsb")
        if COPY_ACT >= 2:
            nc.scalar.copy(out=on_sb, in_=oac_n)
        else:
            nc.vector.tensor_copy(out=on_sb, in_=oac_n)
        return (bh, lc, oe_sb, on_sb)

    # Flatten all (bh, lc, js) into one global software-pipelined stream:
    # QK/exp for step i issue SKEW steps ahead of AV for step i, across
    # chunk boundaries, so the in-order PE queue never drains on the
    # exp(last)->mask->AV(last)->copy chain at a chunk end.
    steps = []  # (bh, lc, js, njs)
    for bh in range(HPC):
        for lc in range(LCH):
            njs = 4 * lc + 4
            for js in range(njs):
                steps.append((bh, lc, js, njs))
    T = len(steps)
    SKEW = 2

    oacs = {}        # (bh, lc) -> (oac_e, oac_n)
    stash = {}       # step idx -> (pt_sb, off)
    copied = {}      # step idx at which to run copies -> chunk key
    epil = {}        # step idx at which to run epilogue -> copies result
    tcount = 0

    for i in range(T + SKEW):
        if i < T:
            bh, lc, js, njs = steps[i]
            qt, kt, ve, vn = tiles[bh]
            lcb = 512 * lc
            off = max(0, 128 * js - lcb)
            sb = 128 * js
            lsl = slice(lcb + off, lcb + 512)
            pt_ps = ps_pt.tile([128, 2, 512], F32, tag="pt")
            nc.tensor.matmul(
                pt_ps[:, 0, off:512],
                kt[0:64, sb : sb + 128],
                qt[0:64, lsl],
                start=True, stop=True, tile_position=(0, 0),
            )
            nc.tensor.matmul(
                pt_ps[:, 1, off:512],
                kt[64:128, sb : sb + 128],
                qt[64:128, lsl],
                start=True, stop=True, tile_position=(64, 0),
            )
            pt_sb = pts.tile([128, 2, 512], BF16, tag="ptsb")
            use_dve = (tcount % DVE_MOD) in DVE_RES
            tcount += 1
            if use_dve:
                nc.vector.tensor_scalar(
                    out=pt_sb[:, :, off:512].bitcast(I16),
                    in0=pt_ps[:, :, off:512],
                    scalar1=stD,
                    scalar2=cdD[:, js : js + 1],
                    op0=MUL,
                    op1=ADD,
                )
            else:
                nc.scalar.activation(
                    out=pt_sb[:, :, off:512],
                    in_=pt_ps[:, :, off:512],
                    func=EXPF,
                    bias=cdelta[:, js : js + 1],
                    scale=st,
                )
            if sb >= lcb:  # diagonal tile: mask l < s
                _meng = nc.gpsimd if MASK_GPS else nc.vector
                for br in range(2):
                    _meng.tensor_mul(
                        pt_sb[:, br, off : off + 128],
                        pt_sb[:, br, off : off + 128],
                        mask,
                    )
            stash[i] = (pt_sb, off)

        j = i - SKEW
        if j < 0:
            continue
        bh, lc, js, njs = steps[j]
        qt, kt, ve, vn = tiles[bh]
        pt_sb, off = stash.pop(j)
        if js == 0:
            oac_e = ps_oac.tile([D + 1, 512], F32, tag="oe")
            oac_n = ps_oac.tile([D + 1, 512], F32, tag="on")
            oacs[(bh, lc)] = (oac_e, oac_n)
        oac_e, oac_n = oacs[(bh, lc)]
        last = js == njs - 1
        nc.tensor.matmul(
            oac_e[:, off:512],
            ve[:, js, :],
            pt_sb[:, 0, off:512],
            start=(js == 0), stop=last,
        )
        nc.tensor.matmul(
            oac_n[:, off:512],
            vn[:, js, :],
            pt_sb[:, 1, off:512],
            start=(js == 0), stop=last,
        )
        if last:
            cres = emit_copies((bh, lc, oac_e, oac_n))
            del oacs[(bh, lc)]
            if j + 2 < T:
                epil[j + 2] = ("w", cres)
            else:
                epilogue_part2(*cres)
        while j in epil:
            kind, arg = epil.pop(j)
            epilogue_part2(*arg)
    for _j in sorted(epil):
        epilogue_part2(*epil[_j][1])


def _host_in_maps(queries, keys, values, keys_null, values_null, tau, delta,
                  soft_mask):
    ident = np.eye(128, dtype=np.float32)
    mask = np.triu(np.ones((128, 128), dtype=np.float32))

    in_maps = []
    for c in range(NCORES):
        b, h0 = c // 4, HPC * (c % 4)
        qkt = np.empty((HPC, 128, 2 * L), ml_dtypes.bfloat16)
        vv = np.empty((HPC, 2, S, D + 1), ml_dtypes.bfloat16)
        for bh in range(HPC):
            h = h0 + bh
            qT = queries[b, :, h, :].T.astype(ml_dtypes.bfloat16)  # [E, L]
            qkt[bh, 0:64, 0:L] = qT
            qkt[bh, 64:128, 0:L] = qT
            qkt[bh, 0:64, L:] = keys[b, :, h, :].T.astype(ml_dtypes.bfloat16)
            qkt[bh, 64:128, L:] = keys_null[b, :, h, :].T.astype(
                ml_dtypes.bfloat16
            )
            vv[bh, 0, :, 0:D] = values[b, :, h, :]
            vv[bh, 0, :, D] = 1.0
            vv[bh, 1, :, 0:D] = values_null[b, :, h, :]
            vv[bh, 1, :, D] = 1.0
        m_t = np.ascontiguousarray(soft_mask[b].reshape(NT, 128).T)
        cdelta = np.ascontiguousarray((delta[b] / 8.0).reshape(NT, 128).T)
        cblob = np.zeros((128, C_NCOL), np.float32)
        cblob[:, C_IDENT : C_IDENT + 128] = ident
        cblob[:, C_CD : C_CD + NT] = cdelta
        cblob[:, C_CDD : C_CDD + NT] = (EXP_A * cdelta + EXP_B).astype(
            np.float32
        )
        cblob[:, C_ST] = tau[b, 0] / 8.0
        cblob[:, C_STD] = EXP_A * tau[b, 0] / 8.0
        mtb = np.empty((128, NT, 2), np.float32)
        mtb[:, :, 0] = m_t
        mtb[:, :, 1] = 1.0 - m_t
        in_maps.append(dict(qkt=qkt, vv=vv, cblob=cblob, mtb=mtb,
                            mask=mask.astype(ml_dtypes.bfloat16)))
    return in_maps


def kernel(queries, keys, values, keys_null, values_null, tau, delta, soft_mask):
    global _NC, LAST_EXEC_NS
    queries = np.asarray(queries, dtype=np.float32)
    keys = np.asarray(keys, dtype=np.float32)
    values = np.asarray(values, dtype=np.float32)
    keys_null = np.asarray(keys_null, dtype=np.float32)
    values_null = np.asarray(values_null, dtype=np.float32)
    tau = np.asarray(tau, dtype=np.float32)
    delta = np.asarray(delta, dtype=np.float32)
    soft_mask = np.asarray(soft_mask, dtype=np.float32)

    if _NC is None:
        _NC = _build()

    in_maps = _host_in_maps(
        queries, keys, values, keys_null, values_null, tau, delta, soft_mask
    )
    res = run_bass_kernel_spmd(
        _NC, in_maps, core_ids=list(range(NCORES)), trace=TRACE
    )
    LAST_EXEC_NS = res.exec_time_ns

    out = np.empty((B, L, H, D), np.float32)
    for c in range(NCORES):
        b, h0 = c // 4, HPC * (c % 4)
        out[b, :, h0 : h0 + HPC, :] = res.results[c]["out"].transpose(1, 0, 2)
    return out
